# revision 1
# baseline (speedup 1.0000x reference)
"""Multi-head attention kernel for 8 Trainium2 NeuronCores.

Problem: B=2, S=2048, D=1024, H=16 heads, head_dim=64 (torch-Linear style
projections: x @ W.T + b).

Sharding: 8 cores = batch (2) x head-groups (4 heads each, 4 groups).
Each core computes, for its batch b and head slice hs..hs+256:
  QT = (w_q_slice/8) @ x_q.T + b_q_slice/8      -> [256, 2048]  (head-dim major)
  KT = w_k_slice @ x_k.T + b_k_slice            -> [256, 2048]
  V  = x_v @ w_v_slice.T + b_v_slice            -> [2048, 256]  (seq major)
  per head h (64 rows of QT/KT, 64+1 cols of V with a ones column):
    S.T chunk = KT_h_chunk.T @ QT_h              (scores transposed, [k,q])
    E = exp(S.T)                                 (no max subtraction; scores are O(10))
    ctxT[0:64] += V_h65.T @ E ; ctxT[64] = rowsum(E)   (ones-column trick)
    ctxT[0:64] *= broadcast(1/ctxT[64])          (PE e0-matmul broadcast)
  out_partial = ctx @ w_o_slice.T               -> [2048, 1024]
Host sums the 4 partials per batch and adds b_o.

Matmul operands are float32r (full PE rate; ~1.5e-4 rel err per contraction).
"""

import numpy as np

B, S, D, H, HD = 2, 2048, 1024, 16, 64
ATTN_ORDER = "h_outer"
EBUFS, SBUFS, CBUFS = 3, 2, 2
N_CORES = 8
GROUPS = 4            # head groups (cores per batch)
HPC = 4               # heads per core
DS = HPC * HD         # 256, d_model slice per core
QB = 512              # q block (matmul moving dim)
NQB = S // QB         # 4
KC = S // 128         # 16 k chunks in attention
DK = D // 128         # 8 contraction chunks in projections

_built = None


def _build(reps=1, phases="ABC"):
    import concourse.bacc as bacc
    import concourse.tile as tile
    from concourse import mybir

    F32 = mybir.dt.float32
    F32R = mybir.dt.float32r
    Exp = mybir.ActivationFunctionType.Exp
    Copy = mybir.ActivationFunctionType.Identity

    nc = bacc.Bacc("TRN2", target_bir_lowering=False, debug=False,
                   num_devices=N_CORES)

    xq = nc.dram_tensor("xq", [D, S], F32R, kind="ExternalInput").ap()
    xk = nc.dram_tensor("xk", [D, S], F32R, kind="ExternalInput").ap()
    xv = nc.dram_tensor("xv", [D, S], F32R, kind="ExternalInput").ap()
    wq = nc.dram_tensor("wq", [D, DS], F32R, kind="ExternalInput").ap()
    wk = nc.dram_tensor("wk", [D, DS], F32R, kind="ExternalInput").ap()
    wv = nc.dram_tensor("wv", [D, DS], F32R, kind="ExternalInput").ap()
    wo = nc.dram_tensor("wo", [DS, D], F32R, kind="ExternalInput").ap()
    bq = nc.dram_tensor("bq", [DS, 1], F32, kind="ExternalInput").ap()
    bk = nc.dram_tensor("bk", [DS, 1], F32, kind="ExternalInput").ap()
    bvb = nc.dram_tensor("bvb", [128, DS], F32, kind="ExternalInput").ap()
    out = nc.dram_tensor("out", [S, D], F32, kind="ExternalOutput").ap()

    with tile.TileContext(nc) as tc, \
         nc.allow_low_precision(reason="float32r matmul operands"):
        for rep in range(reps):
            _emit(nc, tc, tile, mybir, F32, F32R, Exp, Copy,
                  xq, xk, xv, wq, wk, wv, wo, bq, bk, bvb, out, rep=rep,
                  phases=phases)
    nc.compile()
    return nc


def _emit(nc, tc, tile, mybir, F32, F32R, Exp, Copy,
          xq, xk, xv, wq, wk, wv, wo, bq, bk, bvb, out, rep=0,
          phases="ABC"):
    from contextlib import ExitStack

    ctx = ExitStack()
    with ctx:
        consts = ctx.enter_context(tc.tile_pool(name=f"consts{rep}", bufs=1))
        wpool = ctx.enter_context(tc.tile_pool(name=f"wpool{rep}", bufs=1))
        persist = ctx.enter_context(tc.tile_pool(name=f"persist{rep}", bufs=1))

        # ---- constants ----
        e0 = consts.tile([128, HD], F32R, name=f"e0_{rep}_{rep}", tag=f"e0_{rep}_{rep}")
        nc.vector.memset(e0.bitcast(F32), 0.0)
        nc.vector.memset(e0[0:1, :].bitcast(F32), 1.0)
        r_tiles = []
        for i in range(2):
            rt = consts.tile([128, QB], F32R, name=f"r{i}_{rep}", tag=f"r{i}_{rep}")
            nc.vector.memset(rt.bitcast(F32), 0.0)
            r_tiles.append(rt)
        bq_t, bk_t = [], []
        for m in range(2):
            t = consts.tile([128, 1], F32, name=f"bq{m}_{rep}", tag=f"bq{m}_{rep}")
            nc.sync.dma_start(out=t, in_=bq[m * 128:(m + 1) * 128, :])
            bq_t.append(t)
            t = consts.tile([128, 1], F32, name=f"bk{m}_{rep}", tag=f"bk{m}_{rep}")
            nc.sync.dma_start(out=t, in_=bk[m * 128:(m + 1) * 128, :])
            bk_t.append(t)
        bvb_t = consts.tile([128, DS], F32, name=f"bvb_{rep}_{rep}", tag=f"bvb_{rep}_{rep}")
        nc.sync.dma_start(out=bvb_t, in_=bvb)

        # ---- weights ----
        wq_t, wk_t, wv_t = [], [], []
        for kc in range(DK):
            for name, src, lst in (("wq", wq, wq_t), ("wk", wk, wk_t),
                                   ("wv", wv, wv_t)):
                t = wpool.tile([128, DS], F32R, name=f"{name}{kc}_{rep}",
                               tag=f"{name}{kc}_{rep}")
                nc.sync.dma_start(out=t, in_=src[kc * 128:(kc + 1) * 128, :])
                lst.append(t)
        wo_t = []
        for kc in range(2):
            t = wpool.tile([128, D], F32R, name=f"wo{kc}_{rep}", tag=f"wo{kc}_{rep}")
            nc.sync.dma_start(out=t, in_=wo[kc * 128:(kc + 1) * 128, :])
            wo_t.append(t)

        # ---- persistent activations ----
        qt_t = [persist.tile([128, S], F32R, name=f"qt{m}_{rep}", tag=f"qt{m}_{rep}")
                for m in range(2)]
        kt_t = [persist.tile([128, S], F32R, name=f"kt{m}_{rep}", tag=f"kt{m}_{rep}")
                for m in range(2)]
        v_t = [persist.tile([128, HPC * (HD + 1)], F32R, name=f"v{m}_{rep}",
                            tag=f"v{m}_{rep}") for m in range(KC)]
        ctxT_t = [persist.tile([128, S], F32R, name=f"ctxT{m}_{rep}",
                               tag=f"ctxT{m}_{rep}") for m in range(2)]

        # ================= phase A: projections =================
        if "A" not in phases:
            return
        with tc.tile_pool(name=f"xp{rep}", bufs=12) as xp, \
             tc.tile_pool(name=f"ppA{rep}", bufs=4, space="PSUM") as ppA, \
             tc.tile_pool(name=f"ppV{rep}", bufs=4, space="PSUM") as ppV:
            # Q and K projections: out.T layout [256, 2048]
            for name, xsrc, w_sb, b_sb, dst in (
                    ("q", xq, wq_t, bq_t, qt_t),
                    ("k", xk, wk_t, bk_t, kt_t)):
                for nh in range(2):  # S halves
                    xh = []
                    for kc in range(DK):
                        t = xp.tile([128, S // 2], F32R,
                                    name=f"x{name}{nh}{kc}", tag=f"x_{rep}")
                        nc.sync.dma_start(
                            out=t,
                            in_=xsrc[kc * 128:(kc + 1) * 128,
                                     nh * (S // 2):(nh + 1) * (S // 2)])
                        xh.append(t)
                    for m in range(2):
                        for n2 in range(2):
                            ps = ppA.tile([128, QB], F32, name=f"psA_{rep}",
                                          tag=f"psA_{rep}")
                            for kc in range(DK):
                                nc.tensor.matmul(
                                    ps[:, :],
                                    w_sb[kc][:, m * 128:(m + 1) * 128],
                                    xh[kc][:, n2 * QB:(n2 + 1) * QB],
                                    start=(kc == 0), stop=(kc == DK - 1))
                            col = (nh * 2 + n2) * QB
                            nc.scalar.activation(
                                dst[m][:, col:col + QB], ps[:, :], Copy,
                                bias=b_sb[m][:, :], scale=1.0)
            # V projection: natural layout [2048, 4*(64+1)] with ones cols
            for nh in range(2):
                xh = []
                for kc in range(DK):
                    t = xp.tile([128, S // 2], F32R, name=f"xv{nh}{kc}_{rep}",
                                tag=f"x_{rep}")
                    nc.sync.dma_start(
                        out=t,
                        in_=xv[kc * 128:(kc + 1) * 128,
                               nh * (S // 2):(nh + 1) * (S // 2)])
                    xh.append(t)
                for ms in range(8):
                    m = nh * 8 + ms
                    ps = ppV.tile([128, DS], F32, name=f"psV_{rep}_{rep}", tag=f"psV_{rep}_{rep}")
                    for kc in range(DK):
                        nc.tensor.matmul(
                            ps[:, :],
                            xh[kc][:, ms * 128:(ms + 1) * 128],
                            wv_t[kc][:, :],
                            start=(kc == 0), stop=(kc == DK - 1))
                    vm = v_t[m].rearrange("p (h c) -> p h c", h=HPC)
                    nc.vector.tensor_add(
                        vm[:, :, 0:64],
                        ps.rearrange("p (h c) -> p h c", h=HPC),
                        bvb_t.rearrange("p (h c) -> p h c", h=HPC))
                    nc.vector.memset(vm[:, :, 64:65].bitcast(F32), 1.0)

        # ================= phase B: attention =================
        if "B" not in phases:
            nc.sync.dma_start(out=out[0:128, 0:S // 2],
                              in_=qt_t[0][:, 0:S // 2].bitcast(F32))
            return
        # loop: (head, q-block-pair) outer, k-chunk middle, 2 q-blocks
        # inner. The kt / v65 stationary operands are loaded once per
        # k-chunk for 2 matmuls each, exp runs once per [128, 2*QB], and
        # the scores psum is double-buffered so exp(kc) overlaps
        # scores(kc+1). PSUM: 2*2 (scores) + 3 (ctx) + 1 (bcast) = 8 banks.
        with tc.tile_pool(name=f"ep{rep}", bufs=EBUFS) as ep, \
             tc.tile_pool(name=f"bcp{rep}", bufs=2) as bcp, \
             tc.tile_pool(name=f"pss{rep}", bufs=2, space="PSUM") as pss, \
             tc.tile_pool(name=f"psc{rep}", bufs=3, space="PSUM") as psc, \
             tc.tile_pool(name=f"psb{rep}", bufs=1, space="PSUM") as psb:
            it = 0
            for h in range(HPC):
                ti, ro = h // 2, (h % 2) * 64
                qh = qt_t[ti][ro:ro + 64, :]
                kh = kt_t[ti][ro:ro + 64, :]
                for qp in range(NQB // 2):
                    ctx_ps = [psc.tile([128, QB], F32, name=f"ctxps_{rep}",
                                       tag=f"ctxps_{rep}")
                              for _ in range(2)]
                    for kc in range(KC):
                        sps = pss.tile([128, 2, QB], F32,
                                       name=f"sps_{rep}", tag=f"sps_{rep}")
                        for j in range(2):
                            qb = qp * 2 + j
                            nc.tensor.matmul(
                                sps[:, j, :],
                                kh[:, kc * 128:(kc + 1) * 128],
                                qh[:, qb * QB:(qb + 1) * QB],
                                start=True, stop=True)
                        e_sb = ep.tile([128, 2, QB], F32R,
                                       name=f"e_{rep}", tag=f"e_{rep}")
                        nc.scalar.activation(e_sb, sps[:, :, :], Exp)
                        for j in range(2):
                            nc.tensor.matmul(
                                ctx_ps[j][0:65, :],
                                v_t[kc][:, h * 65:(h + 1) * 65],
                                e_sb[:, j, :],
                                start=(kc == 0), stop=(kc == KC - 1))
                    for j in range(2):
                        qb = qp * 2 + j
                        rt = r_tiles[it % len(r_tiles)]
                        it += 1
                        nc.vector.reciprocal(rt[0:1, :],
                                             ctx_ps[j][64:65, :])
                        bps = psb.tile([64, QB], F32, name=f"bps_{rep}",
                                       tag=f"bps_{rep}")
                        nc.tensor.matmul(bps[0:64, :], e0[:, :], rt[:, :],
                                         start=True, stop=True)
                        bsb = bcp.tile([64, QB], F32, name=f"bsb_{rep}",
                                       tag=f"bsb_{rep}")
                        nc.vector.tensor_copy(bsb, bps[0:64, :])
                        nc.vector.tensor_mul(
                            ctxT_t[ti][ro:ro + 64, qb * QB:(qb + 1) * QB],
                            ctx_ps[j][0:64, :], bsb)

        # ================= phase C: output projection =================
        if "C" not in phases:
            nc.sync.dma_start(out=out[0:128, 0:S // 2],
                              in_=ctxT_t[0][:, 0:S // 2].bitcast(F32))
            return
        with tc.tile_pool(name=f"op{rep}", bufs=4) as op, \
             tc.tile_pool(name=f"pso{rep}", bufs=4, space="PSUM") as pso:
            for m in range(KC):
                for n2 in range(2):
                    ps = pso.tile([128, QB], F32, name=f"psO_{rep}_{rep}", tag=f"psO_{rep}_{rep}")
                    for kc in range(2):
                        nc.tensor.matmul(
                            ps[:, :],
                            ctxT_t[kc][:, m * 128:(m + 1) * 128],
                            wo_t[kc][:, n2 * QB:(n2 + 1) * QB],
                            start=(kc == 0), stop=(kc == 1))
                    ot = op.tile([128, QB], F32, name=f"ot_{rep}_{rep}", tag=f"ot_{rep}_{rep}")
                    if (m + n2) % 2 == 0:
                        nc.vector.tensor_copy(ot, ps[:, :])
                    else:
                        nc.scalar.copy(ot, ps[:, :])
                    nc.sync.dma_start(
                        out=out[m * 128:(m + 1) * 128,
                                n2 * QB:(n2 + 1) * QB],
                        in_=ot)


def _in_maps(q, k, v, w_q, b_q, w_k, b_k, w_v, b_v, w_o):
    scale = 1.0 / np.sqrt(HD)
    wqT = np.ascontiguousarray(w_q.T * scale)      # [D, D]
    wkT = np.ascontiguousarray(w_k.T)
    wvT = np.ascontiguousarray(w_v.T)
    maps = []
    for c in range(N_CORES):
        b, g = c // GROUPS, c % GROUPS
        hs = g * DS
        maps.append({
            "xq": np.ascontiguousarray(q[b].T),
            "xk": np.ascontiguousarray(k[b].T),
            "xv": np.ascontiguousarray(v[b].T),
            "wq": np.ascontiguousarray(wqT[:, hs:hs + DS]),
            "wk": np.ascontiguousarray(wkT[:, hs:hs + DS]),
            "wv": np.ascontiguousarray(wvT[:, hs:hs + DS]),
            "wo": np.ascontiguousarray(w_o[:, hs:hs + DS].T),
            "bq": (b_q[hs:hs + DS] * scale).reshape(DS, 1).copy(),
            "bk": b_k[hs:hs + DS].reshape(DS, 1).copy(),
            "bvb": np.broadcast_to(b_v[hs:hs + DS], (128, DS)).copy(),
        })
    return maps


def kernel(q, k, v, w_q, b_q, w_k, b_k, w_v, b_v, w_o, b_o):
    global _built
    arrs = [np.asarray(a, dtype=np.float32)
            for a in (q, k, v, w_q, b_q, w_k, b_k, w_v, b_v, w_o)]
    q, k, v, w_q, b_q, w_k, b_k, w_v, b_v, w_o = arrs
    b_o = np.asarray(b_o, dtype=np.float32)
    if _built is None:
        _built = _build()
    from concourse import bass2jax
    results = bass2jax.run_bass_via_pjrt(
        _built, _in_maps(q, k, v, w_q, b_q, w_k, b_k, w_v, b_v, w_o),
        n_cores=N_CORES)
    o = np.zeros((B, S, D), np.float32)
    for c in range(N_CORES):
        o[c // GROUPS] += results[c]["out"]
    o += b_o
    return o



# revision 35
# speedup vs baseline: 2.8139x; 2.8139x over previous
"""Multi-head attention kernel for 8 Trainium2 NeuronCores.

Problem: B=2, S=2048, D=1024, H=16 heads, head_dim=64 (torch-Linear style
projections: x @ W.T + b).

Sharding: 8 cores = batch (2) x head-groups (4 heads each, 4 groups).
Each core computes, for its batch b and head slice hs..hs+256:
  KT = w_k_slice @ x_k.T + b_k_slice            -> [256, 2048]  (head-dim major)
  V  = x_v @ w_v_slice.T + b_v_slice            -> [2048, 4*(64+1)] with ones col
  QT = (w_q_slice/8) @ x_q.T + b_q_slice/8      -> [256, 2048]
  per (head h, q-block of 512):
    S.T chunk = KT_h_chunk.T @ QT_h             (scores transposed, [k,q])
    E = exp(S.T)                                (no max subtraction; scores O(5))
    ctxT[0:64] += V_h65.T @ E ; ctxT[64] = rowsum(E)  (ones-column trick)
    ctxT[0:64] *= broadcast(1/ctxT[64])         (PE e0-matmul broadcast)
  out_partial = ctx @ w_o_slice.T               -> [2048, 1024] (bf16)
Host sums the 4 partials per batch and adds b_o.

All matmul operands are bf16 (full PE rate); accumulation in fp32 PSUM.
Engine balance: ACT does only exp (the attention-phase bottleneck); DVE does
all PSUM->SBUF copies/bias adds and softmax normalization.

The attention phase is one flat software-pipelined stream over 128
(q-block, head, k-pair) slots: scores run 2 slots ahead of the ctx matmuls
so the in-order PE queue never blocks on exp; the normalization matmul is
deferred one slot past its block; late QT projection tiles and output
projection tiles of the previous q-block are woven into fixed slots so the
PE stays busy while ACT streams exps.
"""

import numpy as np

B, S, D, H, HD = 2, 2048, 1024, 16, 64
N_CORES = 8
GROUPS = 4            # head groups (cores per batch)
HPC = 4               # heads per core
DS = HPC * HD         # 256, d_model slice per core
QB = 512              # q block (matmul moving dim)
DK = D // 128         # 8 contraction chunks in projections
KC = S // 128         # 16 k chunks in attention
NQB = S // QB         # 4 q blocks
NKP = KC // 2         # 8 k-pair slots per (head, q-block)
CB = 2                # x column blocks (S halves)

_built = None


def _build(reps=1, phases="ABC"):
    import concourse.bacc as bacc
    import concourse.tile as tile
    from concourse import mybir

    F32 = mybir.dt.float32
    F32R = mybir.dt.float32r
    BF16 = mybir.dt.bfloat16
    FP8 = mybir.dt.float8e4
    Exp = mybir.ActivationFunctionType.Exp

    nc = bacc.Bacc("TRN2", target_bir_lowering=False, debug=False,
                   num_devices=N_CORES)

    xq = nc.dram_tensor("xq", [D, S], BF16, kind="ExternalInput").ap()
    xk = nc.dram_tensor("xk", [D, S], BF16, kind="ExternalInput").ap()
    xv = nc.dram_tensor("xv", [D, S], BF16, kind="ExternalInput").ap()
    wq = nc.dram_tensor("wq", [D, DS], BF16, kind="ExternalInput").ap()
    wk = nc.dram_tensor("wk", [D, DS], BF16, kind="ExternalInput").ap()
    wv = nc.dram_tensor("wv", [D, DS], BF16, kind="ExternalInput").ap()
    wo = nc.dram_tensor("wo", [DS, D], BF16, kind="ExternalInput").ap()
    bias = nc.dram_tensor("bias", [128, 4 + DS], F32,
                          kind="ExternalInput").ap()
    out = nc.dram_tensor("out", [S, D], BF16, kind="ExternalOutput").ap()

    with tile.TileContext(nc) as tc, \
         nc.allow_low_precision(reason="bf16/fp8 matmul operands"):
        for rep in range(reps):
            _emit(nc, tc, tile, mybir, F32, F32R, BF16, FP8, Exp,
                  xq, xk, xv, wq, wk, wv, wo, bias, out, rep=rep,
                  phases=phases)
    nc.compile()
    return nc


def _emit(nc, tc, tile, mybir, F32, F32R, BF16, FP8, Exp,
          xq, xk, xv, wq, wk, wv, wo, bias, out, rep=0, phases="ABC"):
    from contextlib import ExitStack
    from collections import deque

    ctx = ExitStack()
    with ctx:
        consts = ctx.enter_context(tc.tile_pool(name=f"consts{rep}", bufs=1))
        wpool = ctx.enter_context(tc.tile_pool(name=f"wpool{rep}", bufs=1))
        persist = ctx.enter_context(tc.tile_pool(name=f"persist{rep}", bufs=1))
        xp = ctx.enter_context(tc.tile_pool(name=f"xp{rep}", bufs=4))
        ep = ctx.enter_context(tc.tile_pool(name=f"ep{rep}", bufs=4))
        bcp = ctx.enter_context(tc.tile_pool(name=f"bcp{rep}", bufs=2))
        op = ctx.enter_context(tc.tile_pool(name=f"op{rep}", bufs=3))
        # PSUM budget (8 banks): pss 2x2 + psc 2x1 + ppj 2x1 = 8
        pss = ctx.enter_context(
            tc.tile_pool(name=f"pss{rep}", bufs=2, space="PSUM"))
        psc = ctx.enter_context(
            tc.tile_pool(name=f"psc{rep}", bufs=2, space="PSUM"))
        ppj = ctx.enter_context(
            tc.tile_pool(name=f"ppj{rep}", bufs=2, space="PSUM"))

        # ---- constants (bias DMA is sequenced into the x stream below) ----
        e0 = consts.tile([128, HD], F32R, name=f"e0_{rep}", tag=f"e0_{rep}")
        nc.vector.memset(e0.bitcast(F32), 0.0)
        nc.vector.memset(e0[0:1, :].bitcast(F32), 1.0)
        r_tiles = []
        for i in range(2):
            rt = consts.tile([128, QB], F32R, name=f"r{i}_{rep}",
                             tag=f"r{i}_{rep}")
            nc.vector.memset(rt.bitcast(F32), 0.0)
            r_tiles.append(rt)
        bias_t = consts.tile([128, 4 + DS], F32, name=f"bias_{rep}",
                             tag=f"bias_{rep}")
        bq_t = [bias_t[:, m:m + 1] for m in range(2)]
        bk_t = [bias_t[:, 2 + m:3 + m] for m in range(2)]
        bvb_t = bias_t[:, 4:4 + DS]

        wk_t = wpool.tile([128, DK, DS], BF16, name=f"wk_{rep}",
                          tag=f"wk_{rep}")
        wv_t = wpool.tile([128, DK, DS], BF16, name=f"wv_{rep}",
                          tag=f"wv_{rep}")
        wq_t = wpool.tile([128, DK, DS], BF16, name=f"wq_{rep}",
                          tag=f"wq_{rep}")
        wo_t = wpool.tile([128, 2, D], BF16, name=f"wo_{rep}", tag=f"wo_{rep}")

        # ---- persistent activations (bf16) ----
        qt_t = [persist.tile([128, S], BF16, name=f"qt{m}_{rep}",
                             tag=f"qt{m}_{rep}") for m in range(2)]
        kt_t = [persist.tile([128, S], BF16, name=f"kt{m}_{rep}",
                             tag=f"kt{m}_{rep}") for m in range(2)]
        v_t = [persist.tile([128, HPC * (HD + 1)], BF16, name=f"v{m}_{rep}",
                            tag=f"v{m}_{rep}") for m in range(KC)]
        ctxT_t = [persist.tile([128, S], BF16, name=f"ctxT{m}_{rep}",
                               tag=f"ctxT{m}_{rep}") for m in range(2)]
        for m in range(KC):
            vm = v_t[m].rearrange("p (h c) -> p h c", h=HPC)
            nc.vector.memset(vm[:, :, HD:HD + 1], 1.0)  # ones column

        # ---- input DMAs, in consumption order ----
        xk_tiles, xv_tiles = [], []

        def _ld(xsrc, lst, name, cb, split=1):
            t = xp.tile([128, DK, S // 2], BF16, name=f"{name}{cb}_{rep}",
                        tag=f"x_{rep}")
            src = xsrc.rearrange("(kc p) s -> p kc s", p=128)
            kstep = DK // split
            for s0 in range(split):
                nc.sync.dma_start(
                    out=t[:, s0 * kstep:(s0 + 1) * kstep, :],
                    in_=src[:, s0 * kstep:(s0 + 1) * kstep,
                            cb * (S // 2):(cb + 1) * (S // 2)])
            lst.append(t)

        # q blocks land as four [128, DK, QB] tiles, block 0 first
        xq_src = xq.rearrange("(kc p) s -> p kc s", p=128)
        xq_tiles = [xp.tile([128, DK, QB], BF16, name=f"xq{blk}_{rep}",
                            tag=f"xq_{rep}") for blk in range(NQB)]

        def _ldq(blk):
            nc.sync.dma_start(out=xq_tiles[blk],
                              in_=xq_src[:, :, blk * QB:(blk + 1) * QB])

        nc.sync.dma_start(out=wk_t,
                          in_=wk.rearrange("(kc p) c -> p kc c", p=128))
        _ld(xk, xk_tiles, "xk", 0, split=2)
        nc.sync.dma_start(out=bias_t, in_=bias)
        _ld(xk, xk_tiles, "xk", 1)
        nc.sync.dma_start(out=wv_t,
                          in_=wv.rearrange("(kc p) c -> p kc c", p=128))
        _ld(xv, xv_tiles, "xv", 0)
        nc.sync.dma_start(out=wq_t,
                          in_=wq.rearrange("(kc p) c -> p kc c", p=128))
        _ldq(0)
        _ld(xv, xv_tiles, "xv", 1)
        nc.sync.dma_start(out=wo_t,
                          in_=wo.rearrange("(kc p) c -> p kc c", p=128))
        for blk in range(1, NQB):
            _ldq(blk)

        if "A" not in phases:
            return

        # ---- projection tile emitters ----
        def kt_tile(m, cb, n2):
            ps = ppj.tile([128, QB], F32, name=f"pj_{rep}", tag=f"pj_{rep}")
            for kc in range(DK):
                nc.tensor.matmul(
                    ps[:, :], wk_t[:, kc, m * 128:(m + 1) * 128],
                    xk_tiles[cb][:, kc, n2 * QB:(n2 + 1) * QB],
                    start=(kc == 0), stop=(kc == DK - 1))
            col = (cb * 2 + n2) * QB
            nc.vector.tensor_scalar_add(kt_t[m][:, col:col + QB], ps[:, :],
                                        bk_t[m])

        def v_tile(cb, ms):
            m = cb * 8 + ms
            ps = ppj.tile([128, QB], F32, name=f"pj_{rep}", tag=f"pj_{rep}")
            for kc in range(DK):
                nc.tensor.matmul(
                    ps[:, 0:DS],
                    xv_tiles[cb][:, kc, ms * 128:(ms + 1) * 128],
                    wv_t[:, kc, :],
                    start=(kc == 0), stop=(kc == DK - 1))
            vm = v_t[m].rearrange("p (h c) -> p h c", h=HPC)
            nc.vector.tensor_add(
                vm[:, :, 0:HD],
                ps[:, 0:DS].rearrange("p (h c) -> p h c", h=HPC),
                bvb_t.rearrange("p (h c) -> p h c", h=HPC))

        def qt_tile(m, blk):
            ps = ppj.tile([128, QB], F32, name=f"pj_{rep}", tag=f"pj_{rep}")
            for kc in range(DK):
                nc.tensor.matmul(
                    ps[:, :], wq_t[:, kc, m * 128:(m + 1) * 128],
                    xq_tiles[blk][:, kc, :],
                    start=(kc == 0), stop=(kc == DK - 1))
            col = blk * QB
            nc.vector.tensor_scalar_add(qt_t[m][:, col:col + QB], ps[:, :],
                                        bq_t[m])

        # ---- upfront projections: KT, V half 0, QT block 0 ----
        # (V half 1 and QT blocks 1-3 are filled into the attention stream)
        for cb in range(CB):
            for m in range(2):
                for n2 in range(2):
                    kt_tile(m, cb, n2)
        for ms in range(8):
            v_tile(0, ms)
        for m in range(2):
            qt_tile(m, 0)

        if "B" not in phases:
            nc.sync.dma_start(out=out[0:128, 0:S // 2],
                              in_=qt_t[0][:, 0:S // 2])
            return

        # ---- output-projection half-tile emitter (fill work) ----
        ot_cur = {}

        def c_half(qb, mq, n2, tail=False):
            row = qb * QB + mq * 128
            if n2 == 0:
                ot_cur[(qb, mq)] = op.tile([128, 2, QB], BF16,
                                           name=f"ot_{rep}", tag=f"ot_{rep}")
            ot = ot_cur[(qb, mq)]
            ps = ppj.tile([128, QB], F32, name=f"pj_{rep}", tag=f"pj_{rep}")
            for kc in range(2):
                nc.tensor.matmul(
                    ps[:, :], ctxT_t[kc][:, row:row + 128],
                    wo_t[:, kc, n2 * QB:(n2 + 1) * QB],
                    start=(kc == 0), stop=(kc == 1))
            if tail and (mq + n2) % 2 == 0:
                nc.scalar.copy(ot[:, n2, :], ps[:, :])  # ACT idle at tail
            else:
                nc.vector.tensor_copy(ot[:, n2, :], ps[:, :])
            if tail:
                nc.sync.dma_start(
                    out=out[row:row + 128, n2 * QB:(n2 + 1) * QB],
                    in_=ot[:, n2, :])
                if n2 == 1:
                    del ot_cur[(qb, mq)]
            elif n2 == 1:
                nc.sync.dma_start(
                    out=out[row:row + 128, :].rearrange(
                        "p (n c) -> p n c", n=2),
                    in_=ot)
                del ot_cur[(qb, mq)]

        # ---- fill schedule: (qb, h, kp) -> list of closures ----
        # Budget ~6.8us of fill per q-block against ~5.9us of PE headroom:
        # qb0: V half 1 (arrival-bound); qb1: QT blk1+blk2; qb2: QT blk3 +
        # C(qb0); qb3: C(qb1)+C(qb2); tail: C(qb3).
        fills = {}
        if "C" in phases:
            for h in range(HPC):
                fills[(2, h, 5)] = [(lambda h=h: c_half(0, h, 0))]
                fills[(2, h, 6)] = [(lambda h=h: c_half(0, h, 1))]
                fills[(3, h, 2)] = [(lambda h=h: c_half(1, h, 0))]
                fills[(3, h, 3)] = [(lambda h=h: c_half(1, h, 1))]
                fills[(3, h, 5)] = [(lambda h=h: c_half(2, h, 0))]
                fills[(3, h, 6)] = [(lambda h=h: c_half(2, h, 1))]
        # V half 1 tiles: woven into qb0/h0 once xv half 1 has landed
        for u in range(8):
            fills.setdefault((0, 0 if u < 6 else 1, (5 + u // 2) % NKP), []
                             ).append(lambda u=u: v_tile(1, u))
        # late QT tiles, each one q-block before first use:
        # blk1 late in qb0 (xq blk1 lands mid-qb0), blk2 in qb1, blk3 in qb2
        for blk in range(1, 4):
            for m in range(2):
                qbf, hf = blk - 1, 2 + m if blk == 1 else m
                fills.setdefault((qbf, hf, 1), []).append(
                    (lambda m=m, blk=blk: qt_tile(m, blk)))

        # ---- flat attention pipeline ----
        steps = [(qb, h, kp) for qb in range(NQB) for h in range(HPC)
                 for kp in range(NKP)]
        e_of = {}
        block_ctx = {}
        pending_norm = deque()
        cur_idx = [0]
        it = 0

        def norm(qb, h):
            nonlocal it
            ti, ro = h // 2, (h % 2) * 64
            ctx_ps = block_ctx.pop((qb, h))
            rt = r_tiles[it % len(r_tiles)]
            it += 1
            nc.vector.reciprocal(rt[0:1, :], ctx_ps[HD:HD + 1, :])
            bps = ppj.tile([128, QB], F32, name=f"pj_{rep}", tag=f"pj_{rep}")
            nc.tensor.matmul(bps[0:HD, :], e0[:, :], rt[:, :],
                             start=True, stop=True)
            bsb = bcp.tile([64, QB], F32, name=f"bsb_{rep}", tag=f"bsb_{rep}")
            nc.vector.tensor_copy(bsb, bps[0:HD, :])
            nc.vector.tensor_mul(
                ctxT_t[ti][ro:ro + 64, qb * QB:(qb + 1) * QB],
                ctx_ps[0:HD, :], bsb)

        def emit_ctx(qb, h, kp):
            if kp == 0:
                block_ctx[(qb, h)] = psc.tile([128, QB], F32,
                                              name=f"cps_{rep}",
                                              tag=f"cps_{rep}")
            ctx_ps = block_ctx[(qb, h)]
            e_sb = e_of.pop((qb, h, kp))
            for i in range(2):
                kc = kp * 2 + i
                nc.tensor.matmul(
                    ctx_ps[0:HD + 1, :],
                    v_t[kc][:, h * (HD + 1):(h + 1) * (HD + 1)],
                    e_sb[:, i, :],
                    start=(kp == 0 and i == 0),
                    stop=(kp == NKP - 1 and i == 1))
            if kp == NKP - 1:
                pending_norm.append((qb, h, cur_idx[0]))

        LAG = 4   # scores/exp run LAG slots ahead of ctx (V arrives late)
        for idx, (qb, h, kp) in enumerate(steps):
            cur_idx[0] = idx
            # scores + exp for this slot
            ti, ro = h // 2, (h % 2) * 64
            qh = qt_t[ti][ro:ro + 64, :]
            kh = kt_t[ti][ro:ro + 64, :]
            sps = pss.tile([128, 2, QB], F32, name=f"sps_{rep}",
                           tag=f"sps_{rep}")
            for i in range(2):
                kc = kp * 2 + i
                nc.tensor.matmul(
                    sps[:, i, :], kh[:, kc * 128:(kc + 1) * 128],
                    qh[:, qb * QB:(qb + 1) * QB], start=True, stop=True)
            e_sb = ep.tile([128, 2, QB], BF16, name=f"e_{rep}",
                           tag=f"e_{rep}")
            nc.scalar.activation(e_sb, sps[:, :, :], Exp)
            e_of[(qb, h, kp)] = e_sb
            # deferred ctx (LAG slots behind)
            if idx >= LAG:
                emit_ctx(*steps[idx - LAG])
            # deferred normalization (>=1 slot past its last ctx)
            if pending_norm and pending_norm[0][2] < idx:
                qn_, hn_, _ = pending_norm.popleft()
                norm(qn_, hn_)
            for f in fills.get((qb, h, kp), ()):
                f()

        # flush
        for j in range(LAG, 0, -1):
            emit_ctx(*steps[-j])
        while pending_norm:
            qn_, hn_, _ = pending_norm.popleft()
            norm(qn_, hn_)
        if "C" in phases:
            for mq in range(HPC):
                c_half(NQB - 1, mq, 0, tail=True)
                c_half(NQB - 1, mq, 1, tail=True)


def _in_maps(q, k, v, w_q, b_q, w_k, b_k, w_v, b_v, w_o):
    import ml_dtypes
    bf16 = ml_dtypes.bfloat16
    scale = 1.0 / np.sqrt(HD)
    wqT = np.ascontiguousarray((w_q.T * scale).astype(bf16))   # [D, D]
    wkT = np.ascontiguousarray(w_k.T.astype(bf16))
    wvT = np.ascontiguousarray(w_v.T.astype(bf16))
    woB = w_o.astype(bf16)
    qb = [np.ascontiguousarray(q[b].T.astype(bf16)) for b in range(B)]
    kb = [np.ascontiguousarray(k[b].T.astype(bf16)) for b in range(B)]
    vb = [np.ascontiguousarray(v[b].T.astype(bf16)) for b in range(B)]
    maps = []
    for c in range(N_CORES):
        b, g = c // GROUPS, c % GROUPS
        hs = g * DS
        bias = np.zeros((128, 4 + DS), np.float32)
        for m in range(2):
            bias[:, m] = b_q[hs + m * 128:hs + (m + 1) * 128] * scale
            bias[:, 2 + m] = b_k[hs + m * 128:hs + (m + 1) * 128]
        bias[:, 4:] = np.broadcast_to(b_v[hs:hs + DS], (128, DS))
        maps.append({
            "xq": qb[b],
            "xk": kb[b],
            "xv": vb[b],
            "wq": np.ascontiguousarray(wqT[:, hs:hs + DS]),
            "wk": np.ascontiguousarray(wkT[:, hs:hs + DS]),
            "wv": np.ascontiguousarray(wvT[:, hs:hs + DS]),
            "wo": np.ascontiguousarray(woB[:, hs:hs + DS].T),
            "bias": bias,
        })
    return maps


def kernel(q, k, v, w_q, b_q, w_k, b_k, w_v, b_v, w_o, b_o):
    global _built
    arrs = [np.asarray(a, dtype=np.float32)
            for a in (q, k, v, w_q, b_q, w_k, b_k, w_v, b_v, w_o)]
    q, k, v, w_q, b_q, w_k, b_k, w_v, b_v, w_o = arrs
    b_o = np.asarray(b_o, dtype=np.float32)
    if _built is None:
        _built = _build()
    from concourse import bass2jax
    results = bass2jax.run_bass_via_pjrt(
        _built, _in_maps(q, k, v, w_q, b_q, w_k, b_k, w_v, b_v, w_o),
        n_cores=N_CORES)
    o = np.zeros((B, S, D), np.float32)
    for c in range(N_CORES):
        o[c // GROUPS] += np.asarray(results[c]["out"], dtype=np.float32)
    o += b_o
    return o
